# revision 22
# baseline (speedup 1.0000x reference)
"""LowPassMSELoss Trainium2 kernel (v3: chunked DMA pipeline, bf16 PE path,
PE warmup, token-matmul WAR funneling).

Math: loss = mean((lfilter(b,a,o) - lfilter(b,a,t))^2)
    = mean(lfilter(b,a,o-t)^2)               [filter is linear]
    = mean(conv(o-t, h)^2)                   [h = impulse response, truncated
                                              to K=128 taps; max pole radius
                                              0.869 -> tail < 2e-8]

Per core (2 rows of T=262144), per row:
  - input arrives in 5 DMAs (natural layout, partition p = contiguous
    2048-sample span): mini f[1920:2048] first (pad source), then f-chunks
    [0:512],[512:1024],[1024:1536],[1536:1920]
  - d = (o - t) cast to bf16 on DVE, per chunk
  - transposes as plain bf16 matmuls (lhsT = d block, rhs = I):
    xb data col 128 + 128*tt + p holds 128-sample block (16p + tt)
  - pad cols [0,128): block 16p-1 for col 128+p; built from the mini chunk
    (tt=15 tile), so conv tile j=0 runs as soon as chunk 0 lands
  - conv tile j: psum[jj,n] = sum_i A[i,jj] xb_cur[i,n]
    + sum_i B[i,jj] xb_prev[i,n] (Toeplitz lhsT from h scaled 16x, bf16)
  - square+reduce per psum tile on ACT in place (scale=1/16 pre-func)
    -> per-partition partials; host sums / (16*262144)

One sync-wait per instruction (HW limit) is maintained by: funneling the
consts DMA through a DVE copy, warming recycled psum banks' ACT ticks into
the PE vector clock via a tiny "token" matmul that reads the accumulator
column ACT wrote, and issuing the output DMA from ACT's own HWDGE queue.
A dozen dummy matmuls at kernel start warm the PE HAM clock gate (4/8 ->
8/8) while the first input DMA is still in flight.
"""

import os
import ml_dtypes
import numpy as np

B, T = 16, 262144
NCORES = 8
ROWS_PER_CORE = B // NCORES          # 2
F = 2048                             # free dim of natural layout (T / 128)
K = 128                              # FIR taps
NJ = F // 512                        # 4 conv tiles per row
XBW = 128 + F                        # xb width (128 pad cols + data)
HSCALE = 16.0                        # keep bf16 taps in normal range
NWU = 12                             # PE warmup matmuls

last_exec_time_ns = None
_CACHE = {}


def _impulse_response(b, a, n):
    """First n samples of the IIR impulse response, float64, DF2T like scipy."""
    b = np.asarray(b, np.float64)
    a = np.asarray(a, np.float64)
    b = b / a[0]
    a = a / a[0]
    order = len(a) - 1
    z = np.zeros(order, np.float64)
    h = np.empty(n, np.float64)
    for i in range(n):
        x = 1.0 if i == 0 else 0.0
        y = b[0] * x + z[0]
        znew = np.empty(order, np.float64)
        znew[: order - 1] = z[1:] + b[1:order] * x - a[1:order] * y
        znew[order - 1] = b[order] * x - a[order] * y
        z = znew
        h[i] = y
    return h


def _toeplitz_lhsts(h):
    """lhsT_A[i,j] = h[j-i] (j>=i), lhsT_B[i,j] = h[128+j-i] (i>j).

    y[128n+j] = sum_{i<=j} h[j-i]*cur[i] + sum_{i>j} h[128+j-i]*prev[i]
    matmul(out, lhsT, rhs): out[j, n] = sum_i lhsT[i, j] * rhs[i, n]
    """
    i = np.arange(K)[:, None]
    j = np.arange(K)[None, :]
    dj = j - i
    A = np.where(dj >= 0, h[np.clip(dj, 0, K - 1)], 0.0)
    Bm = np.where(dj < 0, h[np.clip(K + dj, 0, K - 1)], 0.0)
    return A, Bm


def _drop_vacuous_self_waits(nc):
    """trn2 codegen allows one sync-wait per instruction; Tile sometimes
    attaches a same-engine self-wait alongside a foreign one.  Engine queues
    issue in order and every same-engine op increments the engine sem, so a
    self-wait whose threshold is already guaranteed by queue position is
    droppable."""
    import copy

    prior_incs = {}
    for f in nc.m.functions:
        for bb in f.blocks:
            new_list = []
            for ins in bb.instructions:
                si = ins.sync_info
                if (
                    si is not None
                    and si.on_wait
                    and len(si.on_wait) > 1
                    and "Drain" in type(ins).__name__
                ):
                    waits = list(si.on_wait)
                    for k, w in enumerate(waits[:-1]):
                        pre = copy.deepcopy(ins)
                        pre.name = f"{ins.name}-w{k}"
                        pre.sync_info = copy.deepcopy(si)
                        pre.sync_info.on_wait = [w]
                        pre.sync_info.on_update = []
                        new_list.append(pre)
                    si.on_wait = [waits[-1]]
                new_list.append(ins)
            bb.instructions = new_list
    for f in nc.m.functions:
        for bb in f.blocks:
            for ins in bb.instructions:
                si = ins.sync_info
                if si is None:
                    continue
                waits = list(si.on_wait or [])
                if len(waits) > 1:
                    kept = []
                    for w in waits:
                        name = getattr(w, "ant_name", "") or ""
                        eng = getattr(getattr(ins, "engine", None), "value", "zz")
                        if (
                            name.startswith(eng)
                            and prior_incs.get(name, 0) >= (w.wait_value or 0)
                        ):
                            continue
                        kept.append(w)
                    si.on_wait = kept
                for u in si.on_update or []:
                    name = getattr(u, "ant_name", "") or ""
                    if name:
                        prior_incs[name] = prior_incs.get(name, 0) + (
                            u.update_value or 1
                        )


def _build_bass():
    import concourse.bass as bass
    import concourse.tile as tile
    from concourse import mybir

    dt = mybir.dt
    nc = bass.Bass(trn_type="TRN2")

    ot_h = nc.dram_tensor(
        "ot", [ROWS_PER_CORE, 2, T], dt.float32, kind="ExternalInput"
    )
    C_h = nc.dram_tensor("consts", [3, K, K], dt.bfloat16, kind="ExternalInput")
    out_h = nc.dram_tensor(
        "partials", [128, ROWS_PER_CORE * NJ], dt.float32, kind="ExternalOutput"
    )

    # ot4[r, p, s, f] = ot[r, s, 2048p + f]
    ot4 = ot_h[:].rearrange("b s (p f) -> b p s f", p=128)

    # per-row f-chunks: (f0, f1, transpose-tile base tt0)
    CH = [(0, 512, 0), (512, 1024, 4), (1024, 1536, 8), (1536, 1920, 12)]

    with tile.TileContext(nc) as tc:
        with (
            tc.tile_pool(name="consts", bufs=1) as consts,
            tc.tile_pool(name="io", bufs=2 * (NJ + 1)) as io_pool,
            tc.tile_pool(name="dpool", bufs=3) as dpool,
            tc.tile_pool(name="xb", bufs=ROWS_PER_CORE) as xbpool,
            tc.tile_pool(name="wu", bufs=1, space="PSUM") as wu_pool,
            tc.tile_pool(name="ptr", bufs=3, space="PSUM") as ptr_pool,
            tc.tile_pool(name="yp", bufs=4, space="PSUM") as y_pool,
            tc.tile_pool(name="outp", bufs=1) as out_pool,
        ):
            # ---- PE HAM warmup: dummy matmuls while the first DMA flies ----
            wu_sb = out_pool.tile([128, 512], dt.bfloat16, tag="wusb")
            nc.vector.memset(wu_sb[:], 0.0)
            wu_ps = wu_pool.tile([128, 512], dt.float32, tag="wu")
            for _ in range(NWU):
                nc.tensor.matmul(
                    wu_ps[:], wu_sb[:, 0:128], wu_sb[:], start=True, stop=True
                )

            # ---- input DMAs: mini (pad source) first, then the chunks ----
            io_tiles = {}
            mini_tiles = {}
            first = True
            for r in range(ROWS_PER_CORE):
                t_mini = io_pool.tile(
                    [128, 2, 128], dt.float32, tag="mini", name="mini"
                )
                nc.sync.dma_start(t_mini[:], ot4[r][:, :, 1920:2048])
                mini_tiles[r] = t_mini
                for ci, (f0, f1, _) in enumerate(CH):
                    t_io = io_pool.tile(
                        [128, 2, f1 - f0], dt.float32, tag="ot", name="ot"
                    )
                    nc.sync.dma_start(t_io[:], ot4[r][:, :, f0:f1])
                    io_tiles[(r, ci)] = t_io
                    if first:
                        c_raw = consts.tile([K, 3, K], dt.bfloat16, tag="Craw")
                        nc.sync.dma_start(
                            c_raw[:], C_h[:].rearrange("c p f -> p c f")
                        )
                        first = False
            # funnel the const-DMA dep through DVE so PE ops wait on one engine
            c_sb = consts.tile([K, 3, K], dt.bfloat16, tag="C")
            nc.vector.tensor_copy(c_sb[:], c_raw[:])
            A_sb = c_sb[:, 0, :]
            B_sb = c_sb[:, 1, :]
            I_sb = c_sb[:, 2, :]

            out_sb = out_pool.tile([128, ROWS_PER_CORE * NJ], dt.float32)

            tile_ct = [0]

            def y_tile():
                py = y_pool.tile([128, 512], dt.float32, tag="y", name="y")
                n = tile_ct[0]
                if n >= 4:
                    # bank recycled; its last reader was the ACT square of
                    # tile n-4, which wrote out_sb col (n-4).  A 1x1 "token"
                    # matmul reading that column pulls the ACT tick into PE's
                    # vector clock, so the real A matmul needs only its DVE
                    # data wait (1 sync-wait HW limit).
                    pc = n - 4
                    nc.tensor.matmul(
                        py[0:1, 0:1],
                        out_sb[:, pc : pc + 1],
                        out_sb[:, pc : pc + 1],
                        start=True,
                        stop=True,
                    )
                tile_ct[0] += 1
                return py

            for r in range(ROWS_PER_CORE):
                xb = xbpool.tile([128, XBW], dt.bfloat16, tag="xb")

                # mini chunk: tt=15 tile -> xb data cols 2048+ AND pad cols.
                # pad col p needs block 16p-1 = tile tt=15 col p-1;
                # col 0 = zeros (zero filter state at row start)
                t_mini = mini_tiles[r]
                d16m = dpool.tile([128, 128], dt.bfloat16, tag="dm", name="dm")
                nc.vector.tensor_sub(d16m[:], t_mini[:, 0, :], t_mini[:, 1, :])
                ptrm = ptr_pool.tile([128, 128], dt.float32, tag="tr", name="trm")
                nc.tensor.matmul(
                    ptrm[:], d16m[:], I_sb[:], start=True, stop=True
                )
                nc.vector.memset(xb[:, 0:1], 0.0)
                nc.vector.tensor_copy(xb[:, 1:128], ptrm[:, 0:127])
                nc.vector.tensor_copy(xb[:, 2048:2176], ptrm[:])

                for ci, (f0, f1, tt0) in enumerate(CH):
                    w = f1 - f0
                    nt = w // 128
                    t_io = io_tiles[(r, ci)]
                    d16 = dpool.tile([128, 512], dt.bfloat16, tag="d")
                    nc.vector.tensor_sub(
                        d16[:, 0:w], t_io[:, 0, :], t_io[:, 1, :]
                    )

                    ptr = ptr_pool.tile([128, 512], dt.float32, tag="tr")
                    for q in range(nt):
                        nc.tensor.matmul(
                            ptr[:, 128 * q : 128 * (q + 1)],
                            d16[:, 128 * q : 128 * (q + 1)],
                            I_sb[:],
                            start=True,
                            stop=True,
                        )
                    nc.vector.tensor_copy(
                        xb[:, 128 + f0 : 128 + f1], ptr[:, 0:w]
                    )

                    # conv tile ci: A on cur 512 cols, B on prev 512 cols.
                    # ci<3 ready now; ci==3 also needs the mini cols (done).
                    j = ci
                    py = y_tile()
                    nc.tensor.matmul(
                        py[:],
                        A_sb[:],
                        xb[:, 128 + 512 * j : 128 + 512 * (j + 1)],
                        start=True,
                        stop=False,
                    )
                    nc.tensor.matmul(
                        py[:],
                        B_sb[:],
                        xb[:, 512 * j : 512 * (j + 1)],
                        start=False,
                        stop=True,
                    )
                    col = NJ * r + j
                    nc.scalar.activation(
                        py[:],
                        py[:],
                        mybir.ActivationFunctionType.Square,
                        scale=1.0 / HSCALE,
                        accum_out=out_sb[:, col : col + 1],
                    )

            # issue from ACT's HWDGE queue: the dep on ACT's accum writes is
            # implicit in program order, keeping this under the 1-wait limit
            nc.scalar.dma_start(out_h[:], out_sb[:])

    _drop_vacuous_self_waits(nc)
    return nc


def kernel(output, target, b, a):
    global last_exec_time_ns
    from concourse.bass_utils import run_bass_kernel_spmd

    output = np.asarray(output, np.float32)
    target = np.asarray(target, np.float32)

    if "nc" not in _CACHE:
        _CACHE["nc"] = _build_bass()
    nc = _CACHE["nc"]

    h = _impulse_response(np.asarray(b, np.float64), np.asarray(a, np.float64), K)
    A_m, B_m = _toeplitz_lhsts(h * HSCALE)
    consts = np.stack([A_m, B_m, np.eye(K)]).astype(ml_dtypes.bfloat16)

    ot = np.stack([output, target], axis=1)  # [B, 2, T]
    in_maps = []
    for c in range(NCORES):
        rows = slice(c * ROWS_PER_CORE, (c + 1) * ROWS_PER_CORE)
        in_maps.append(
            {
                "ot": np.ascontiguousarray(ot[rows]),
                "consts": consts,
            }
        )

    res = run_bass_kernel_spmd(
        nc,
        in_maps,
        core_ids=list(range(NCORES)),
        trace=bool(int(os.environ.get("LP_TRACE", "0"))),
    )
    last_exec_time_ns = res.exec_time_ns

    total = np.float64(0.0)
    for r in res.results:
        total += r["partials"].astype(np.float64).sum()
    # squares are descaled by 1/HSCALE inside the ACT (scale applies pre-func)
    return np.float32(total / (B * T))


# revision 27
# speedup vs baseline: 1.0564x; 1.0564x over previous
"""LowPassMSELoss Trainium2 kernel (v3: chunked DMA pipeline, bf16 PE path,
PE warmup, token-matmul WAR funneling).

Math: loss = mean((lfilter(b,a,o) - lfilter(b,a,t))^2)
    = mean(lfilter(b,a,o-t)^2)               [filter is linear]
    = mean(conv(o-t, h)^2)                   [h = impulse response, truncated
                                              to K=128 taps; max pole radius
                                              0.869 -> tail < 2e-8]

Per core (2 rows of T=262144), per row:
  - input arrives in 5 DMAs (natural layout, partition p = contiguous
    2048-sample span): mini f[1920:2048] first (pad source), then f-chunks
    [0:512],[512:1024],[1024:1536],[1536:1920]
  - d = (o - t) cast to bf16 on DVE, per chunk
  - transposes as plain bf16 matmuls (lhsT = d block, rhs = I):
    xb data col 128 + 128*tt + p holds 128-sample block (16p + tt)
  - pad cols [0,128): block 16p-1 for col 128+p; built from the mini chunk
    (tt=15 tile), so conv tile j=0 runs as soon as chunk 0 lands
  - conv tile j: psum[jj,n] = sum_i A[i,jj] xb_cur[i,n]
    + sum_i B[i,jj] xb_prev[i,n] (Toeplitz lhsT from h scaled 16x, bf16)
  - square+reduce per psum tile on ACT in place (scale=1/16 pre-func)
    -> per-partition partials; host sums / (16*262144)

One sync-wait per instruction (HW limit) is maintained by: funneling the
consts DMA through a DVE copy, warming recycled psum banks' ACT ticks into
the PE vector clock via a tiny "token" matmul that reads the accumulator
column ACT wrote, and issuing the output DMA from ACT's own HWDGE queue.
A dozen dummy matmuls at kernel start warm the PE HAM clock gate (4/8 ->
8/8) while the first input DMA is still in flight.
"""

import os
import ml_dtypes
import numpy as np

B, T = 16, 262144
NCORES = 8
ROWS_PER_CORE = B // NCORES          # 2
F = 2048                             # free dim of natural layout (T / 128)
K = 128                              # FIR taps
NJ = F // 512                        # 4 conv tiles per row
XBW = 128 + F                        # xb width (128 pad cols + data)
HSCALE = 16.0                        # keep bf16 taps in normal range
NWU = 6                              # PE warmup matmuls

last_exec_time_ns = None
_CACHE = {}


def _impulse_response(b, a, n):
    """First n samples of the IIR impulse response, float64, DF2T like scipy."""
    b = np.asarray(b, np.float64)
    a = np.asarray(a, np.float64)
    b = b / a[0]
    a = a / a[0]
    order = len(a) - 1
    z = np.zeros(order, np.float64)
    h = np.empty(n, np.float64)
    for i in range(n):
        x = 1.0 if i == 0 else 0.0
        y = b[0] * x + z[0]
        znew = np.empty(order, np.float64)
        znew[: order - 1] = z[1:] + b[1:order] * x - a[1:order] * y
        znew[order - 1] = b[order] * x - a[order] * y
        z = znew
        h[i] = y
    return h


def _toeplitz_lhsts(h):
    """lhsT_A[i,j] = h[j-i] (j>=i), lhsT_B[i,j] = h[128+j-i] (i>j).

    y[128n+j] = sum_{i<=j} h[j-i]*cur[i] + sum_{i>j} h[128+j-i]*prev[i]
    matmul(out, lhsT, rhs): out[j, n] = sum_i lhsT[i, j] * rhs[i, n]
    """
    i = np.arange(K)[:, None]
    j = np.arange(K)[None, :]
    dj = j - i
    A = np.where(dj >= 0, h[np.clip(dj, 0, K - 1)], 0.0)
    Bm = np.where(dj < 0, h[np.clip(K + dj, 0, K - 1)], 0.0)
    return A, Bm


def _drop_vacuous_self_waits(nc):
    """trn2 codegen allows one sync-wait per instruction; Tile sometimes
    attaches a same-engine self-wait alongside a foreign one.  Engine queues
    issue in order and every same-engine op increments the engine sem, so a
    self-wait whose threshold is already guaranteed by queue position is
    droppable."""
    import copy

    prior_incs = {}
    for f in nc.m.functions:
        for bb in f.blocks:
            new_list = []
            for ins in bb.instructions:
                si = ins.sync_info
                if (
                    si is not None
                    and si.on_wait
                    and len(si.on_wait) > 1
                    and "Drain" in type(ins).__name__
                ):
                    waits = list(si.on_wait)
                    for k, w in enumerate(waits[:-1]):
                        pre = copy.deepcopy(ins)
                        pre.name = f"{ins.name}-w{k}"
                        pre.sync_info = copy.deepcopy(si)
                        pre.sync_info.on_wait = [w]
                        pre.sync_info.on_update = []
                        new_list.append(pre)
                    si.on_wait = [waits[-1]]
                new_list.append(ins)
            bb.instructions = new_list
    for f in nc.m.functions:
        for bb in f.blocks:
            for ins in bb.instructions:
                si = ins.sync_info
                if si is None:
                    continue
                waits = list(si.on_wait or [])
                if len(waits) > 1:
                    kept = []
                    for w in waits:
                        name = getattr(w, "ant_name", "") or ""
                        eng = getattr(getattr(ins, "engine", None), "value", "zz")
                        if (
                            name.startswith(eng)
                            and prior_incs.get(name, 0) >= (w.wait_value or 0)
                        ):
                            continue
                        kept.append(w)
                    si.on_wait = kept
                for u in si.on_update or []:
                    name = getattr(u, "ant_name", "") or ""
                    if name:
                        prior_incs[name] = prior_incs.get(name, 0) + (
                            u.update_value or 1
                        )


def _build_bass():
    import concourse.bass as bass
    import concourse.tile as tile
    from concourse import mybir

    dt = mybir.dt
    nc = bass.Bass(trn_type="TRN2")

    ot_h = nc.dram_tensor(
        "ot", [ROWS_PER_CORE, 2, T], dt.float32, kind="ExternalInput"
    )
    C_h = nc.dram_tensor("consts", [3, K, K], dt.bfloat16, kind="ExternalInput")
    out_h = nc.dram_tensor(
        "partials", [128, ROWS_PER_CORE * NJ], dt.float32, kind="ExternalOutput"
    )

    # ot4[r, p, s, f] = ot[r, s, 2048p + f]
    ot4 = ot_h[:].rearrange("b s (p f) -> b p s f", p=128)

    # per-row f-chunks in DMA order: chunk 3 (tt 12-15) first so the pad
    # (tt=15) exists before conv tile j=0; then 0, 1, 2.  (f0, f1, j)
    CH = [(1536, 2048, 3), (0, 512, 0), (512, 1024, 1), (1024, 1536, 2)]

    with tile.TileContext(nc) as tc:
        with (
            tc.tile_pool(name="consts", bufs=1) as consts,
            tc.tile_pool(name="io", bufs=2 * (NJ + 1)) as io_pool,
            tc.tile_pool(name="dpool", bufs=3) as dpool,
            tc.tile_pool(name="xb", bufs=ROWS_PER_CORE) as xbpool,
            tc.tile_pool(name="wu", bufs=1, space="PSUM") as wu_pool,
            tc.tile_pool(name="ptr", bufs=3, space="PSUM") as ptr_pool,
            tc.tile_pool(name="yp", bufs=4, space="PSUM") as y_pool,
            tc.tile_pool(name="outp", bufs=1) as out_pool,
        ):
            # ---- PE HAM warmup: dummy matmuls while the first DMA flies ----
            wu_sb = out_pool.tile([128, 512], dt.bfloat16, tag="wusb")
            nc.vector.memset(wu_sb[:], 0.0)
            wu_ps = wu_pool.tile([128, 512], dt.float32, tag="wu")
            for _ in range(NWU):
                nc.tensor.matmul(
                    wu_ps[:], wu_sb[:, 0:128], wu_sb[:], start=True, stop=True
                )

            # ---- input DMAs: per row chunk 3 first, then 0, 1, 2 ----
            io_tiles = {}
            n_dma = 0
            for r in range(ROWS_PER_CORE):
                for ci, (f0, f1, _) in enumerate(CH):
                    t_io = io_pool.tile(
                        [128, 2, 512], dt.float32, tag="ot", name="ot"
                    )
                    nc.sync.dma_start(t_io[:], ot4[r][:, :, f0:f1])
                    io_tiles[(r, ci)] = t_io
                    n_dma += 1
                    if n_dma == 2:
                        c_raw = consts.tile([K, 3, K], dt.bfloat16, tag="Craw")
                        nc.sync.dma_start(
                            c_raw[:], C_h[:].rearrange("c p f -> p c f")
                        )
            # funnel the const-DMA dep through DVE so PE ops wait on one engine
            c_sb = consts.tile([K, 3, K], dt.bfloat16, tag="C")
            nc.vector.tensor_copy(c_sb[:], c_raw[:])
            A_sb = c_sb[:, 0, :]
            B_sb = c_sb[:, 1, :]
            I_sb = c_sb[:, 2, :]

            out_sb = out_pool.tile([128, ROWS_PER_CORE * NJ], dt.float32)

            tile_ct = [0]

            def y_tile():
                py = y_pool.tile([128, 512], dt.float32, tag="y", name="y")
                n = tile_ct[0]
                if n >= 4:
                    # bank recycled; its last reader was the ACT square of
                    # tile n-4, which wrote out_sb col (n-4).  A 1x1 "token"
                    # matmul reading that column pulls the ACT tick into PE's
                    # vector clock, so the real A matmul needs only its DVE
                    # data wait (1 sync-wait HW limit).
                    pc = n - 4
                    nc.tensor.matmul(
                        py[0:1, 0:1],
                        out_sb[:, pc : pc + 1],
                        out_sb[:, pc : pc + 1],
                        start=True,
                        stop=True,
                    )
                tile_ct[0] += 1
                return py

            def conv_tile(r, j, xb):
                # B first: its deps (prev+cur chunk casts) are a superset of
                # A's, so the pair only becomes ready together and the
                # scheduler keeps it adjacent -- split psum accumulation
                # pairs (another group's start/stop in between) have been
                # observed to corrupt the accumulation on HW.
                py = y_tile()
                nc.tensor.matmul(
                    py[:],
                    B_sb[:],
                    xb[:, 512 * j : 512 * (j + 1)],
                    start=True,
                    stop=False,
                )
                nc.tensor.matmul(
                    py[:],
                    A_sb[:],
                    xb[:, 128 + 512 * j : 128 + 512 * (j + 1)],
                    start=False,
                    stop=True,
                )
                col = NJ * r + j
                nc.scalar.activation(
                    py[:],
                    py[:],
                    mybir.ActivationFunctionType.Square,
                    scale=1.0 / HSCALE,
                    accum_out=out_sb[:, col : col + 1],
                )

            for r in range(ROWS_PER_CORE):
                xb = xbpool.tile([128, XBW], dt.bfloat16, tag="xb")

                for ci, (f0, f1, j) in enumerate(CH):
                    t_io = io_tiles[(r, ci)]
                    d16 = dpool.tile([128, 512], dt.bfloat16, tag="d")
                    nc.vector.tensor_sub(d16[:], t_io[:, 0, :], t_io[:, 1, :])

                    ptr = ptr_pool.tile([128, 512], dt.float32, tag="tr")
                    for q in range(4):
                        nc.tensor.matmul(
                            ptr[:, 128 * q : 128 * (q + 1)],
                            d16[:, 128 * q : 128 * (q + 1)],
                            I_sb[:],
                            start=True,
                            stop=True,
                        )
                    nc.vector.tensor_copy(
                        xb[:, 128 + f0 : 128 + f1], ptr[:]
                    )

                    if j == 3:
                        # chunk 3 lands first and carries tile tt=15 at
                        # ptr[:, 384:512]: pad col p = block 16p-1 = tt15
                        # col p-1; col 0 = zeros (zero state at row start)
                        nc.vector.memset(xb[:, 0:1], 0.0)
                        nc.vector.tensor_copy(
                            xb[:, 1:128], ptr[:, 384 : 384 + 127]
                        )
                    else:
                        # conv tile j: A on cur 512 cols, B on prev 512
                        conv_tile(r, j, xb)
                        if j == 2:
                            # chunk 2 is the row's last arrival; tile 3's
                            # B operand (cols 1536:2048) is now complete
                            conv_tile(r, 3, xb)

            # issue from ACT's HWDGE queue: the dep on ACT's accum writes is
            # implicit in program order, keeping this under the 1-wait limit
            nc.scalar.dma_start(out_h[:], out_sb[:])

    _drop_vacuous_self_waits(nc)
    return nc


def kernel(output, target, b, a):
    global last_exec_time_ns
    from concourse.bass_utils import run_bass_kernel_spmd

    output = np.asarray(output, np.float32)
    target = np.asarray(target, np.float32)

    if "nc" not in _CACHE:
        _CACHE["nc"] = _build_bass()
    nc = _CACHE["nc"]

    h = _impulse_response(np.asarray(b, np.float64), np.asarray(a, np.float64), K)
    A_m, B_m = _toeplitz_lhsts(h * HSCALE)
    consts = np.stack([A_m, B_m, np.eye(K)]).astype(ml_dtypes.bfloat16)

    ot = np.stack([output, target], axis=1)  # [B, 2, T]
    in_maps = []
    for c in range(NCORES):
        rows = slice(c * ROWS_PER_CORE, (c + 1) * ROWS_PER_CORE)
        in_maps.append(
            {
                "ot": np.ascontiguousarray(ot[rows]),
                "consts": consts,
            }
        )

    res = run_bass_kernel_spmd(
        nc,
        in_maps,
        core_ids=list(range(NCORES)),
        trace=bool(int(os.environ.get("LP_TRACE", "0"))),
    )
    last_exec_time_ns = res.exec_time_ns

    total = np.float64(0.0)
    for r in res.results:
        total += r["partials"].astype(np.float64).sum()
    # squares are descaled by 1/HSCALE inside the ACT (scale applies pre-func)
    return np.float32(total / (B * T))


# revision 29
# speedup vs baseline: 1.0723x; 1.0151x over previous
"""LowPassMSELoss Trainium2 kernel (v3: chunked DMA pipeline, bf16 PE path,
PE warmup, token-matmul WAR funneling).

Math: loss = mean((lfilter(b,a,o) - lfilter(b,a,t))^2)
    = mean(lfilter(b,a,o-t)^2)               [filter is linear]
    = mean(conv(o-t, h)^2)                   [h = impulse response, truncated
                                              to K=128 taps; max pole radius
                                              0.869 -> tail < 2e-8]

Per core (2 rows of T=262144), per row:
  - input arrives in 5 DMAs (natural layout, partition p = contiguous
    2048-sample span): mini f[1920:2048] first (pad source), then f-chunks
    [0:512],[512:1024],[1024:1536],[1536:1920]
  - d = (o - t) cast to bf16 on DVE, per chunk
  - transposes as plain bf16 matmuls (lhsT = d block, rhs = I):
    xb data col 128 + 128*tt + p holds 128-sample block (16p + tt)
  - pad cols [0,128): block 16p-1 for col 128+p; built from the mini chunk
    (tt=15 tile), so conv tile j=0 runs as soon as chunk 0 lands
  - conv tile j: psum[jj,n] = sum_i A[i,jj] xb_cur[i,n]
    + sum_i B[i,jj] xb_prev[i,n] (Toeplitz lhsT from h scaled 16x, bf16)
  - square+reduce per psum tile on ACT in place (scale=1/16 pre-func)
    -> per-partition partials; host sums / (16*262144)

One sync-wait per instruction (HW limit) is maintained by: funneling the
consts DMA through a DVE copy, warming recycled psum banks' ACT ticks into
the PE vector clock via a tiny "token" matmul that reads the accumulator
column ACT wrote, and issuing the output DMA from ACT's own HWDGE queue.
A dozen dummy matmuls at kernel start warm the PE HAM clock gate (4/8 ->
8/8) while the first input DMA is still in flight.
"""

import os
import ml_dtypes
import numpy as np

B, T = 16, 262144
NCORES = 8
ROWS_PER_CORE = B // NCORES          # 2
F = 2048                             # free dim of natural layout (T / 128)
K = 128                              # FIR taps
NJ = F // 512                        # 4 conv tiles per row
XBW = 128 + F                        # xb width (128 pad cols + data)
HSCALE = 16.0                        # keep bf16 taps in normal range
NWU = 8                              # PE warmup matmuls

last_exec_time_ns = None
_CACHE = {}


def _impulse_response(b, a, n):
    """First n samples of the IIR impulse response, float64, DF2T like scipy."""
    b = np.asarray(b, np.float64)
    a = np.asarray(a, np.float64)
    b = b / a[0]
    a = a / a[0]
    order = len(a) - 1
    z = np.zeros(order, np.float64)
    h = np.empty(n, np.float64)
    for i in range(n):
        x = 1.0 if i == 0 else 0.0
        y = b[0] * x + z[0]
        znew = np.empty(order, np.float64)
        znew[: order - 1] = z[1:] + b[1:order] * x - a[1:order] * y
        znew[order - 1] = b[order] * x - a[order] * y
        z = znew
        h[i] = y
    return h


def _toeplitz_lhsts(h):
    """lhsT_A[i,j] = h[j-i] (j>=i), lhsT_B[i,j] = h[128+j-i] (i>j).

    y[128n+j] = sum_{i<=j} h[j-i]*cur[i] + sum_{i>j} h[128+j-i]*prev[i]
    matmul(out, lhsT, rhs): out[j, n] = sum_i lhsT[i, j] * rhs[i, n]
    """
    i = np.arange(K)[:, None]
    j = np.arange(K)[None, :]
    dj = j - i
    A = np.where(dj >= 0, h[np.clip(dj, 0, K - 1)], 0.0)
    Bm = np.where(dj < 0, h[np.clip(K + dj, 0, K - 1)], 0.0)
    return A, Bm


def _drop_vacuous_self_waits(nc):
    """trn2 codegen allows one sync-wait per instruction; Tile sometimes
    attaches a same-engine self-wait alongside a foreign one.  Engine queues
    issue in order and every same-engine op increments the engine sem, so a
    self-wait whose threshold is already guaranteed by queue position is
    droppable."""
    import copy

    prior_incs = {}
    for f in nc.m.functions:
        for bb in f.blocks:
            new_list = []
            for ins in bb.instructions:
                si = ins.sync_info
                if (
                    si is not None
                    and si.on_wait
                    and len(si.on_wait) > 1
                    and "Drain" in type(ins).__name__
                ):
                    waits = list(si.on_wait)
                    for k, w in enumerate(waits[:-1]):
                        pre = copy.deepcopy(ins)
                        pre.name = f"{ins.name}-w{k}"
                        pre.sync_info = copy.deepcopy(si)
                        pre.sync_info.on_wait = [w]
                        pre.sync_info.on_update = []
                        new_list.append(pre)
                    si.on_wait = [waits[-1]]
                new_list.append(ins)
            bb.instructions = new_list
    for f in nc.m.functions:
        for bb in f.blocks:
            for ins in bb.instructions:
                si = ins.sync_info
                if si is None:
                    continue
                waits = list(si.on_wait or [])
                if len(waits) > 1:
                    kept = []
                    for w in waits:
                        name = getattr(w, "ant_name", "") or ""
                        eng = getattr(getattr(ins, "engine", None), "value", "zz")
                        if (
                            name.startswith(eng)
                            and prior_incs.get(name, 0) >= (w.wait_value or 0)
                        ):
                            continue
                        kept.append(w)
                    si.on_wait = kept
                for u in si.on_update or []:
                    name = getattr(u, "ant_name", "") or ""
                    if name:
                        prior_incs[name] = prior_incs.get(name, 0) + (
                            u.update_value or 1
                        )


def _build_bass():
    import concourse.bass as bass
    import concourse.tile as tile
    from concourse import mybir

    dt = mybir.dt
    nc = bass.Bass(trn_type="TRN2")

    ot_h = nc.dram_tensor(
        "ot", [ROWS_PER_CORE, 2, T], dt.float32, kind="ExternalInput"
    )
    C_h = nc.dram_tensor("consts", [3, K, K], dt.bfloat16, kind="ExternalInput")
    out_h = nc.dram_tensor(
        "partials", [128, ROWS_PER_CORE * NJ], dt.float32, kind="ExternalOutput"
    )

    # ot4[r, p, s, f] = ot[r, s, 2048p + f]
    ot4 = ot_h[:].rearrange("b s (p f) -> b p s f", p=128)

    # per-row f-chunks in DMA order: chunk 3 (tt 12-15) first so the pad
    # (tt=15) exists before conv tile j=0; then 0, 1, 2.  (f0, f1, j)
    CH = [(1536, 2048, 3), (0, 512, 0), (512, 1024, 1), (1024, 1536, 2)]

    with tile.TileContext(nc) as tc:
        with (
            tc.tile_pool(name="consts", bufs=1) as consts,
            tc.tile_pool(name="io", bufs=2 * (NJ + 1)) as io_pool,
            tc.tile_pool(name="dpool", bufs=3) as dpool,
            tc.tile_pool(name="xb", bufs=ROWS_PER_CORE) as xbpool,
            tc.tile_pool(name="wu", bufs=1, space="PSUM") as wu_pool,
            tc.tile_pool(name="ptr", bufs=3, space="PSUM") as ptr_pool,
            tc.tile_pool(name="yp", bufs=4, space="PSUM") as y_pool,
            tc.tile_pool(name="outp", bufs=1) as out_pool,
        ):
            # ---- PE HAM warmup: dummy matmuls while the first DMA flies ----
            wu_sb = out_pool.tile([128, 512], dt.bfloat16, tag="wusb")
            nc.vector.memset(wu_sb[:], 0.0)
            wu_ps = wu_pool.tile([128, 512], dt.float32, tag="wu")
            for _ in range(NWU):
                nc.tensor.matmul(
                    wu_ps[:], wu_sb[:, 0:128], wu_sb[:], start=True, stop=True
                )

            # ---- input DMAs: consts first (the DVE funnel copy must run
            # before the subs start occupying DVE), then per row chunk 3
            # first, then 0, 1, 2 ----
            c_raw = consts.tile([K, 3, K], dt.bfloat16, tag="Craw")
            nc.sync.dma_start(c_raw[:], C_h[:].rearrange("c p f -> p c f"))
            # funnel the const-DMA dep through DVE so PE ops wait on one engine
            c_sb = consts.tile([K, 3, K], dt.bfloat16, tag="C")
            nc.vector.tensor_copy(c_sb[:], c_raw[:])

            io_tiles = {}
            for r in range(ROWS_PER_CORE):
                for ci, (f0, f1, _) in enumerate(CH):
                    t_io = io_pool.tile(
                        [128, 2, 512], dt.float32, tag="ot", name="ot"
                    )
                    nc.sync.dma_start(t_io[:], ot4[r][:, :, f0:f1])
                    io_tiles[(r, ci)] = t_io
            A_sb = c_sb[:, 0, :]
            B_sb = c_sb[:, 1, :]
            I_sb = c_sb[:, 2, :]

            out_sb = out_pool.tile([128, ROWS_PER_CORE * NJ], dt.float32)

            tile_ct = [0]

            def y_tile():
                py = y_pool.tile([128, 512], dt.float32, tag="y", name="y")
                n = tile_ct[0]
                if n >= 4:
                    # bank recycled; its last reader was the ACT square of
                    # tile n-4, which wrote out_sb col (n-4).  A 1x1 "token"
                    # matmul reading that column pulls the ACT tick into PE's
                    # vector clock, so the real A matmul needs only its DVE
                    # data wait (1 sync-wait HW limit).
                    pc = n - 4
                    nc.tensor.matmul(
                        py[0:1, 0:1],
                        out_sb[:, pc : pc + 1],
                        out_sb[:, pc : pc + 1],
                        start=True,
                        stop=True,
                    )
                tile_ct[0] += 1
                return py

            def conv_tile(r, j, xb):
                # B first: its deps (prev+cur chunk casts) are a superset of
                # A's, so the pair only becomes ready together and the
                # scheduler keeps it adjacent -- split psum accumulation
                # pairs (another group's start/stop in between) have been
                # observed to corrupt the accumulation on HW.
                py = y_tile()
                nc.tensor.matmul(
                    py[:],
                    B_sb[:],
                    xb[:, 512 * j : 512 * (j + 1)],
                    start=True,
                    stop=False,
                )
                nc.tensor.matmul(
                    py[:],
                    A_sb[:],
                    xb[:, 128 + 512 * j : 128 + 512 * (j + 1)],
                    start=False,
                    stop=True,
                )
                col = NJ * r + j
                nc.scalar.activation(
                    py[:],
                    py[:],
                    mybir.ActivationFunctionType.Square,
                    scale=1.0 / HSCALE,
                    accum_out=out_sb[:, col : col + 1],
                )

            for r in range(ROWS_PER_CORE):
                xb = xbpool.tile([128, XBW], dt.bfloat16, tag="xb")

                for ci, (f0, f1, j) in enumerate(CH):
                    t_io = io_tiles[(r, ci)]
                    d16 = dpool.tile([128, 512], dt.bfloat16, tag="d")
                    nc.vector.tensor_sub(d16[:], t_io[:, 0, :], t_io[:, 1, :])

                    ptr = ptr_pool.tile([128, 512], dt.float32, tag="tr")
                    for q in range(4):
                        nc.tensor.matmul(
                            ptr[:, 128 * q : 128 * (q + 1)],
                            d16[:, 128 * q : 128 * (q + 1)],
                            I_sb[:],
                            start=True,
                            stop=True,
                        )
                    nc.vector.tensor_copy(
                        xb[:, 128 + f0 : 128 + f1], ptr[:]
                    )

                    if j == 3:
                        # chunk 3 lands first and carries tile tt=15 at
                        # ptr[:, 384:512]: pad col p = block 16p-1 = tt15
                        # col p-1; col 0 = zeros (zero state at row start)
                        nc.vector.memset(xb[:, 0:1], 0.0)
                        nc.vector.tensor_copy(
                            xb[:, 1:128], ptr[:, 384 : 384 + 127]
                        )
                    else:
                        # conv tile j: A on cur 512 cols, B on prev 512
                        conv_tile(r, j, xb)
                        if j == 2:
                            # chunk 2 is the row's last arrival; tile 3's
                            # B operand (cols 1536:2048) is now complete
                            conv_tile(r, 3, xb)

            # issue from ACT's HWDGE queue: the dep on ACT's accum writes is
            # implicit in program order, keeping this under the 1-wait limit
            nc.scalar.dma_start(out_h[:], out_sb[:])

    _drop_vacuous_self_waits(nc)
    return nc


def kernel(output, target, b, a):
    global last_exec_time_ns
    from concourse.bass_utils import run_bass_kernel_spmd

    output = np.asarray(output, np.float32)
    target = np.asarray(target, np.float32)

    if "nc" not in _CACHE:
        _CACHE["nc"] = _build_bass()
    nc = _CACHE["nc"]

    h = _impulse_response(np.asarray(b, np.float64), np.asarray(a, np.float64), K)
    A_m, B_m = _toeplitz_lhsts(h * HSCALE)
    consts = np.stack([A_m, B_m, np.eye(K)]).astype(ml_dtypes.bfloat16)

    ot = np.stack([output, target], axis=1)  # [B, 2, T]
    in_maps = []
    for c in range(NCORES):
        rows = slice(c * ROWS_PER_CORE, (c + 1) * ROWS_PER_CORE)
        in_maps.append(
            {
                "ot": np.ascontiguousarray(ot[rows]),
                "consts": consts,
            }
        )

    res = run_bass_kernel_spmd(
        nc,
        in_maps,
        core_ids=list(range(NCORES)),
        trace=bool(int(os.environ.get("LP_TRACE", "0"))),
    )
    last_exec_time_ns = res.exec_time_ns

    total = np.float64(0.0)
    for r in res.results:
        total += r["partials"].astype(np.float64).sum()
    # squares are descaled by 1/HSCALE inside the ACT (scale applies pre-func)
    return np.float32(total / (B * T))
